# revision 1
# baseline (speedup 1.0000x reference)
"""MultiHeadAttention (B=2, S=2048, D=1024, 16 heads, causal, torch-.view head
split) on 8 TRN2 NeuronCores.

Sharding: core c handles batch b = c//4 and heads [4g, 4g+4) with g = c%4
(head h only touches token rows [128h, 128(h+1)) of its batch, so each core
needs just 512 rows of q/k/v). Wp is row-sharded by head; each core returns a
partial (2048, 1024) output and the host sums the 4 partials per batch.

Layout notes:
- Head h's (2048, 64) matrices come from the (128 tokens x 1024 cols) block
  via s = 16*t + c, d = col%64, c = col//64. On-chip we keep head-space
  sequence order PERMUTED within each 128-tile: w = 8*c + t_lo (t = 8*j+t_lo),
  which makes all gather DMAs 32B-contiguous while preserving the causal
  block structure. The final output DMA un-permutes.
- Everything is bf16 except PSUM accumulation, softmax denominators and the
  final output (f32).
"""

import numpy as np
import ml_dtypes
from contextlib import ExitStack

import concourse.bass as bass
import concourse.tile as tile
from concourse import bacc, mybir
from concourse.bass_utils import run_bass_kernel_spmd
from concourse.masks import make_identity

F32 = mybir.dt.float32
F16 = mybir.dt.float16
F16_NP = np.float16
BF16 = mybir.dt.bfloat16
BF16_NP = ml_dtypes.bfloat16

B, S, D, NH, HD = 2, 2048, 1024, 16, 64
HPC = 4          # heads per core
ROWS = 512       # token rows per core
N_CORES = 8
EXP_FN = mybir.ActivationFunctionType.Exp


def _perm_mask_np():
    """(128,128) bf16 mask in permuted within-tile coords: mask[wk, wq] = 1
    iff s(wq) >= s(wk), with s(w) = 16*(w%8) + w//8."""
    w = np.arange(128)
    s = 16 * (w % 8) + w // 8
    m = (s[None, :] >= s[:, None]).astype(np.float32)
    return m.astype(BF16_NP)


_PROGRAM = None


def _build_program(debug_dump=False, trunc=None):
    nc = bacc.Bacc("TRN2", target_bir_lowering=False, debug=False)

    qT_d = nc.dram_tensor("qT", [D, ROWS], F16, kind="ExternalInput").ap()
    kT_d = nc.dram_tensor("kT", [D, ROWS], F16, kind="ExternalInput").ap()
    vT_d = nc.dram_tensor("vT", [D, ROWS], BF16, kind="ExternalInput").ap()
    Wq_d = nc.dram_tensor("Wq", [D, D], F16, kind="ExternalInput").ap()
    Wk_d = nc.dram_tensor("Wk", [D, D], F16, kind="ExternalInput").ap()
    Wv_d = nc.dram_tensor("Wv", [D, D], BF16, kind="ExternalInput").ap()
    Wp_d = nc.dram_tensor("Wp", [HPC * HD, D], BF16, kind="ExternalInput").ap()
    mask_d = nc.dram_tensor("mask", [128, 128], BF16, kind="ExternalInput").ap()
    out_d = nc.dram_tensor("out", [S, D], F32, kind="ExternalOutput").ap()
    dbg = {}
    if debug_dump:
        for nm, shape, dt in [
            ("dQT", [128, 8, ROWS], F16),
            ("dKT", [128, 8, ROWS], F16),
            ("dVT", [128, 8, ROWS], BF16),
            ("dQhT", [128, 2, 16, 16, 8], F16),
            ("dKhT", [128, 2, 16, 16, 8], F16),
            ("dVnat", [128, HPC, 16, HD + 1], BF16),
            ("dattT2", [128, 2, 16, 128], BF16),
            ("dPT", [128, 16, ROWS], BF16),
        ]:
            dbg[nm] = nc.dram_tensor(nm, shape, dt, kind="ExternalOutput").ap()

    with tile.TileContext(nc) as tc:
        with ExitStack() as ctx:
            # ---------------- persistent tensors ----------------
            pers = ctx.enter_context(tc.tile_pool(name="pers", bufs=1))
            phaseA = ctx.enter_context(tc.tile_pool(name="phaseA", bufs=1))
            # projected X^T, block layout: [p, dblk, t] = X^T[128*dblk+p, t]
            QT_sb = phaseA.tile([128, 8, ROWS], F16)
            KT_sb = phaseA.tile([128, 8, ROWS], F16)
            VT_sb = phaseA.tile([128, 8, ROWS], BF16)
            # head-gathered, pair-packed: [64*(h%2)+d, h//2, j, c, t_lo]
            QhT = pers.tile([128, 2, 16, 16, 8], F16)
            KhT = pers.tile([128, 2, 16, 16, 8], F16)
            V_pre = phaseA.tile([128, 2, 16, 16, 8], BF16)
            # partition-half-swapped copies of the projections
            QT_sw = phaseA.tile([128, 8, ROWS], F16)
            KT_sw = phaseA.tile([128, 8, ROWS], F16)
            VT_sw = phaseA.tile([128, 8, ROWS], BF16)
            # V natural per head + ones column: [w, hl, j, 0:65]
            V_nat = pers.tile([128, HPC, 16, HD + 1], BF16)
            # att^T pair-packed for Wp: [64*(h%2)+d, h//2, qt, wq]
            attT2 = pers.tile([128, 2, 16, 128], BF16)
            Wp_sb = pers.tile([128, 2, D], BF16)
            mask_t = pers.tile([128, 128], BF16)
            ident = pers.tile([128, 128], BF16)
            make_identity(nc, ident)

            nc.sync.dma_start(out=mask_t, in_=mask_d)
            nc.sync.dma_start(
                out=Wp_sb, in_=Wp_d.rearrange("(a p) e -> p a e", p=128)
            )
            nc.gpsimd.memset(V_nat[:, :, :, HD : HD + 1], 1.0)

            # ---------------- projections (order: v, k, q) ----------------
            ps512 = ctx.enter_context(
                tc.tile_pool(name="ps512", bufs=4, space="PSUM")
            )
            pst = ctx.enter_context(tc.tile_pool(name="pst", bufs=2, space="PSUM"))

            def gather_batch(dst, src_sb, src_sw):
                """Head gather dst[64par+d, hp, j, c, tl] =
                src[64(c%2)+d, c//2, 128hl + 8j + tl] as partition-aligned
                ENGINE copies (multi-dim free APs), reading the half-swapped
                copy when par != c%2. 8 copies per tensor, split DVE/GpSimd."""
                i = 0
                for hl in range(HPC):
                    par, hp = hl % 2, hl // 2
                    po = 64 * par
                    for c0 in range(2):
                        srct = src_sb if par == c0 else src_sw
                        inv = srct[
                            po : po + 64, :, 128 * hl : 128 * (hl + 1)
                        ].rearrange("d a (j w) -> d j a w", w=8)
                        outv = dst[po : po + 64, hp].rearrange(
                            "d j (cc c2) w -> d j cc c2 w", c2=2
                        )[:, :, :, c0, :]
                        eng = nc.vector if i % 2 == 0 else nc.gpsimd
                        eng.tensor_copy(outv, inv)
                        i += 1

            with tc.tile_pool(name="xin", bufs=1) as xin_pool, tc.tile_pool(
                name="wcol", bufs=3
            ) as w_pool:
                proj = [
                    (qT_d, Wq_d, QT_sb, QT_sw, F16),
                    (kT_d, Wk_d, KT_sb, KT_sw, F16),
                    (vT_d, Wv_d, VT_sb, VT_sw, BF16),
                ]
                for xd, wd, xt_out, xt_sw, xdt in proj:
                    x_in = xin_pool.tile([128, 8, ROWS], xdt, tag="x_in")
                    nc.sync.dma_start(
                        out=x_in, in_=xd.rearrange("(a p) t -> p a t", p=128)
                    )
                    for dblk in range(8):
                        wcol = w_pool.tile([128, 8, 128], xdt, tag="wcol")
                        nc.sync.dma_start(
                            out=wcol,
                            in_=wd[:, 128 * dblk : 128 * (dblk + 1)].rearrange(
                                "(a p) d -> p a d", p=128
                            ),
                        )
                        psum = ps512.tile([128, ROWS], F32, tag="ps512")
                        for mt in range(8):
                            nc.tensor.matmul(
                                psum,
                                lhsT=wcol[:, mt, :],
                                rhs=x_in[:, mt, :],
                                start=(mt == 0),
                                stop=(mt == 7),
                            )
                        if xt_out is VT_sb:
                            nc.vector.tensor_copy(xt_out[:, dblk, :], psum)
                        else:
                            nc.scalar.copy(xt_out[:, dblk, :], psum)
                    # half-swap copy, then gathers (engine copies)
                    nc.sync.dma_start(out=xt_sw[0:64], in_=xt_out[64:128])
                    nc.sync.dma_start(out=xt_sw[64:128], in_=xt_out[0:64])
                    if xt_out is VT_sb:
                        gather_batch(V_pre, VT_sb, VT_sw)
                        for hl in range(HPC):
                            hp, ho = hl // 2, (hl % 2) * 64
                            for j in range(16):
                                ps_v = pst.tile([128, HD], BF16, tag="pst")
                                nc.tensor.transpose(
                                    ps_v,
                                    V_pre[ho : ho + 64, hp, j, :, :],
                                    ident[ho : ho + 64, ho : ho + 64],
                                )
                                nc.vector.tensor_copy(
                                    V_nat[:, hl, j, 0:HD], ps_v
                                )
                    elif xt_out is KT_sb:
                        gather_batch(KhT, KT_sb, KT_sw)
                    else:
                        gather_batch(QhT, QT_sb, QT_sw)

            if debug_dump:
                nc.sync.dma_start(out=dbg["dQT"], in_=QT_sb)
                nc.sync.dma_start(out=dbg["dKT"], in_=KT_sb)
                nc.sync.dma_start(out=dbg["dVT"], in_=VT_sb)
                nc.sync.dma_start(out=dbg["dQhT"], in_=QhT)
                nc.sync.dma_start(out=dbg["dKhT"], in_=KhT)
                nc.sync.dma_start(out=dbg["dVnat"], in_=V_nat)
            skip_attn = trunc == "gather"

            # ---------------- attention + output projection ----------------
            if not skip_attn:
                att_ps = ctx.enter_context(
                    tc.tile_pool(name="attps", bufs=2, space="PSUM")
                )
                PT_arr = []
                for i in range(2):
                    pt_half_a = pers.tile([128, 16, ROWS], BF16, tag=f"pt{i}a")
                    pt_half_b = pers.tile([128, 16, ROWS], BF16, tag=f"pt{i}b")
                    PT_arr.append([pt_half_a, pt_half_b])
                sm_pool = ctx.enter_context(tc.tile_pool(name="small", bufs=8))
                an_pool = ctx.enter_context(tc.tile_pool(name="attn", bufs=4))
                out_pool = ctx.enter_context(tc.tile_pool(name="outt", bufs=4))

            def st_exp(qc, hpair, kt, phase):
                """S^T matmul + exp (+ diagonal mask) for both heads of the
                pair into PT_arr[phase % 2] slots (kt, half)."""
                qoff = max(0, 128 * kt - 512 * qc)
                pts = []
                for half in range(2):
                    ho = 64 * half
                    psS = ps512.tile([128, ROWS], F32, tag="ps512")
                    nc.tensor.matmul(
                        psS[:, qoff:512],
                        lhsT=KhT[ho : ho + 64, hpair, kt, :, :],
                        rhs=QhT[
                            ho : ho + 64, hpair,
                            4 * qc + qoff // 128 : 4 * (qc + 1), :, :,
                        ],
                        start=True,
                        stop=True,
                    )
                    PT = PT_arr[phase % 2][half][:, kt, :]
                    nc.scalar.activation(PT[:, qoff:512], psS[:, qoff:512], EXP_FN)
                    if kt >= 4 * qc:  # diagonal tile
                        nc.vector.tensor_mul(
                            PT[:, qoff : qoff + 128],
                            PT[:, qoff : qoff + 128],
                            mask_t,
                        )
                    pts.append(PT)
                return pts

            n_qc = 0 if skip_attn else (1 if trunc == "attn1" else 4)
            for qc in range(n_qc):
                for hpair in range(2):
                    phase = 2 * qc + hpair
                    pts = {}
                    for kt in range(4 * qc + 1):
                        pts[kt] = st_exp(qc, hpair, kt, phase)
                    for s in range(4):
                        if s > 0:
                            pts[4 * qc + s] = st_exp(qc, hpair, 4 * qc + s, phase)
                        attn2 = an_pool.tile([128, 128], BF16, tag="attn2")
                        for half in range(2):
                            hl = 2 * hpair + half
                            acc = att_ps.tile([128, HD + 1], F32, tag="acc")
                            for kt in range(4 * qc + s + 1):
                                nc.tensor.matmul(
                                    acc,
                                    lhsT=pts[kt][half][:, 128 * s : 128 * (s + 1)],
                                    rhs=V_nat[:, hl, kt, :],
                                    start=(kt == 0),
                                    stop=(kt == 4 * qc + s),
                                )
                            recip = sm_pool.tile([128, 1], F32, tag="recip")
                            nc.vector.reciprocal(recip, acc[:, HD : HD + 1])
                            nc.vector.tensor_scalar_mul(
                                attn2[:, 64 * half : 64 * (half + 1)],
                                acc[:, 0:HD],
                                recip,
                            )
                        ps_t = pst.tile([128, 128], BF16, tag="pst")
                        nc.tensor.transpose(ps_t, attn2, ident)
                        nc.vector.tensor_copy(attT2[:, hpair, 4 * qc + s, :], ps_t)
                # Wp for this chunk's 4 q-tiles
                for s in range(4):
                    qt = 4 * qc + s
                    for ec in range(2):
                        ps_o = ps512.tile([128, ROWS], F32, tag="ps512")
                        for pair in range(2):
                            nc.tensor.matmul(
                                ps_o,
                                lhsT=attT2[:, pair, qt, :],
                                rhs=Wp_sb[:, pair, 512 * ec : 512 * (ec + 1)],
                                start=(pair == 0),
                                stop=(pair == 1),
                            )
                        out_t = out_pool.tile([128, ROWS], F32, tag="out_t")
                        nc.vector.tensor_copy(out_t, ps_o)
                        # un-permute rows: partition w=8c+tl -> row 16*tl+c.
                        # DRAM-side AP traversal (c outer, tl inner) matches
                        # the SBUF partition order w = 8c+tl.
                        dst = out_d[
                            128 * qt : 128 * (qt + 1), 512 * ec : 512 * (ec + 1)
                        ].rearrange("(tl c) e -> c tl e", tl=8)
                        nc.sync.dma_start(out=dst, in_=out_t)
            if debug_dump and not skip_attn:
                nc.sync.dma_start(out=dbg["dattT2"], in_=attT2)
                nc.sync.dma_start(out=dbg["dPT"], in_=PT_arr[1][0])

    nc.compile()
    return nc


def get_program(debug_dump=False, trunc=None):
    global _PROGRAM
    if _PROGRAM is None:
        _PROGRAM = _build_program(debug_dump, trunc)
    return _PROGRAM


def make_in_maps(q, k, v, Wq, Wk, Wv, Wp):
    mask = _perm_mask_np()
    Wq_b = np.asarray(Wq, np.float32).astype(F16_NP)
    Wk_b = np.asarray(Wk, np.float32).astype(F16_NP)
    Wv_b = np.asarray(Wv, np.float32).astype(BF16_NP)
    Wp_f = np.asarray(Wp, np.float32)
    in_maps = []
    for core in range(N_CORES):
        b, g = divmod(core, 4)
        rows = slice(ROWS * g, ROWS * (g + 1))
        in_maps.append(
            {
                "qT": np.ascontiguousarray(
                    np.asarray(q[b], np.float32)[rows].T
                ).astype(F16_NP),
                "kT": np.ascontiguousarray(
                    np.asarray(k[b], np.float32)[rows].T
                ).astype(F16_NP),
                "vT": np.ascontiguousarray(
                    np.asarray(v[b], np.float32)[rows].T
                ).astype(BF16_NP),
                "Wq": Wq_b,
                "Wk": Wk_b,
                "Wv": Wv_b,
                "Wp": np.ascontiguousarray(
                    Wp_f[HPC * HD * g : HPC * HD * (g + 1)]
                ).astype(BF16_NP),
                "mask": mask,
            }
        )
    return in_maps


def kernel(q, k, v, Wq, Wk, Wv, Wp, _trace=False, _trace_kwargs=None):
    nc = get_program()
    in_maps = make_in_maps(q, k, v, Wq, Wk, Wv, Wp)
    res = run_bass_kernel_spmd(
        nc,
        in_maps,
        core_ids=list(range(N_CORES)),
        trace=_trace,
        **(_trace_kwargs or {}),
    )
    outs = [res.results[c]["out"] for c in range(N_CORES)]
    full = np.stack(
        [
            outs[0] + outs[1] + outs[2] + outs[3],
            outs[4] + outs[5] + outs[6] + outs[7],
        ]
    ).astype(np.float32)
    if _trace:
        kernel._last_result = res
    return full



# revision 31
# speedup vs baseline: 1.1149x; 1.1149x over previous
"""MultiHeadAttention (B=2, S=2048, D=1024, 16 heads, causal, torch-.view head
split) on 8 TRN2 NeuronCores.

Sharding: core c handles batch b = c//4 and heads [4g, 4g+4) with g = c%4
(head h only touches token rows [128h, 128(h+1)) of its batch, so each core
needs just 512 rows of q/k/v). Wp is row-sharded by head; each core returns a
partial (2048, 1024) output (bf16) and the host sums the 4 partials per batch
in f32.

Layout notes:
- Head h's (2048, 64) matrices come from the (128 tokens x 1024 cols) block
  via s = 16*t + c, d = col%64, c = col//64. On-chip we keep head-space
  sequence order PERMUTED within each 128-tile: w = 8*c + t_lo (t = 8*j+t_lo),
  which makes all gather copies contiguous while preserving the causal
  block structure. The final output DMA un-permutes.
- Scheduling: attention q-chunks run in DESCENDING order so the scalar
  engine's exp work (the secondary bottleneck) starts as early as possible,
  overlapped with the V projection which is interleaved into the PE stream.
- S^T matmuls for the two heads of a pair use disjoint partition halves and
  execute concurrently on the PE (row_grp h0/h64); their exps are fused into
  one 2-bank-wide activation.
"""

import numpy as np
import ml_dtypes
from contextlib import ExitStack

import concourse.bass as bass
import concourse.tile as tile
from concourse import bacc, mybir
from concourse.bass_utils import run_bass_kernel_spmd
from concourse.masks import make_identity

F32 = mybir.dt.float32
F16 = mybir.dt.float16
F16_NP = np.float16
BF16 = mybir.dt.bfloat16
BF16_NP = ml_dtypes.bfloat16

B, S, D, NH, HD = 2, 2048, 1024, 16, 64
HPC = 4          # heads per core
ROWS = 512       # token rows per core
N_CORES = 8
EXP_FN = mybir.ActivationFunctionType.Exp


def _perm_mask_np():
    """(128,128) bf16 mask in permuted within-tile coords: mask[wk, wq] = 1
    iff s(wq) >= s(wk), with s(w) = 16*(w%8) + w//8."""
    w = np.arange(128)
    s = 16 * (w % 8) + w // 8
    m = (s[None, :] >= s[:, None]).astype(np.float32)
    return m.astype(BF16_NP)


_PROGRAM = None


def _build_program(debug_dump=False, trunc=None):
    nc = bacc.Bacc("TRN2", target_bir_lowering=False, debug=False)

    qT_d = nc.dram_tensor("qT", [D, ROWS], F16, kind="ExternalInput").ap()
    kT_d = nc.dram_tensor("kT", [D, ROWS], F16, kind="ExternalInput").ap()
    vT_d = nc.dram_tensor("vT", [D, ROWS], BF16, kind="ExternalInput").ap()
    Wq_d = nc.dram_tensor("Wq", [D, D], F16, kind="ExternalInput").ap()
    Wk_d = nc.dram_tensor("Wk", [D, D], F16, kind="ExternalInput").ap()
    Wv_d = nc.dram_tensor("Wv", [D, D], BF16, kind="ExternalInput").ap()
    Wp_d = nc.dram_tensor("Wp", [HPC * HD, D], BF16, kind="ExternalInput").ap()
    mask_d = nc.dram_tensor("mask", [128, 128], BF16, kind="ExternalInput").ap()
    out_d = nc.dram_tensor("out", [S, D], BF16, kind="ExternalOutput").ap()
    dbg = {}
    if debug_dump:
        for nm, shape, dt in [
            ("dKT", [128, 8, ROWS], F16),
            ("dQhT", [128, 2, 16, 16, 8], F16),
            ("dKhT", [128, 2, 16, 16, 8], F16),
            ("dVnat", [128, HPC, 16, HD + 1], BF16),
            ("dPT0", [128, 2, 16, ROWS], BF16),
            ("dPT1", [128, 2, 16, ROWS], BF16),
            ("dattT2", [128, 2, 16, 128], BF16),
        ]:
            dbg[nm] = nc.dram_tensor(nm, shape, dt, kind="ExternalOutput").ap()

    with tile.TileContext(nc) as tc:
        with ExitStack() as ctx:
            # ---------------- persistent SBUF ----------------
            pers = ctx.enter_context(tc.tile_pool(name="pers", bufs=1))
            # projected X^T, block layout: [p, dblk, t] = X^T[128*dblk+p, t]
            QT_sb = pers.tile([128, 8, ROWS], F16, tag="QT_sb")
            KT_sb = pers.tile([128, 8, ROWS], F16, tag="KT_sb")
            VT_sb = pers.tile([128, 8, ROWS], BF16, tag="VT_sb")
            # partition-half-swapped copies
            QT_sw = pers.tile([128, 8, ROWS], F16, tag="QT_sw")
            KT_sw = pers.tile([128, 8, ROWS], F16, tag="KT_sw")
            VT_sw = pers.tile([128, 8, ROWS], BF16, tag="VT_sw")
            # head-gathered, pair-packed: [64*(h%2)+d, h//2, j, c, t_lo]
            QhT = pers.tile([128, 2, 16, 16, 8], F16, tag="QhT")
            KhT = pers.tile([128, 2, 16, 16, 8], F16, tag="KhT")
            V_pre = pers.tile([128, 2, 16, 16, 8], BF16, tag="V_pre")
            # V natural per head + ones column: [w, hl, j, 0:65]
            V_nat = pers.tile([128, HPC, 16, HD + 1], BF16, tag="V_nat")
            # exp'd S^T, double-buffered by qc parity: [kpos, half, kt, q]
            PT = [
                pers.tile(
                    [128, 2, 16, ROWS], BF16, tag=f"PT{i}", name=f"PT{i}"
                )
                for i in range(2)
            ]
            # att^T pair-packed for Wp: [64*(h%2)+d, h//2, qt, wq]
            attT2 = pers.tile([128, 2, 16, 128], BF16, tag="attT2")
            Wp_sb = pers.tile([128, 2, D], BF16, tag="Wp_sb")
            mask_t = pers.tile([128, 128], BF16, tag="mask_t")
            ident = pers.tile([128, 128], BF16, tag="ident")
            make_identity(nc, ident)

            # Non-weight DMAs go through the Pool engine's DGE so the SP
            # hardware-queue rotation carries only the 24 wcol DMAs: with
            # wcol bufs=8 each slot-reuse DMA lands on the same queue as its
            # predecessor, keeping sync-wait counts within the HW limit.
            nc.sync.dma_start(out=mask_t, in_=mask_d)
            nc.sync.dma_start(
                out=Wp_sb, in_=Wp_d.rearrange("(a p) e -> p a e", p=128)
            )
            nc.gpsimd.memset(V_nat[:, :, :, HD : HD + 1], 1.0)

            # ---------------- PSUM pools ----------------
            # psA: [128,512] f32 single-bank tiles: projections, S^T halves,
            # out-proj. psPV: one bank, 4 rotating [128,65] accumulators.
            # psT: one bank, transposes (V groups + att^T).
            psA = ctx.enter_context(tc.tile_pool(name="psA", bufs=5, space="PSUM"))
            psPV_pool = ctx.enter_context(
                tc.tile_pool(name="psPV", bufs=2, space="PSUM")
            )
            psT = ctx.enter_context(tc.tile_pool(name="psT", bufs=1, space="PSUM"))

            xin_pool = ctx.enter_context(tc.tile_pool(name="xin", bufs=2))
            w_pool = ctx.enter_context(tc.tile_pool(name="wcol", bufs=8))
            an_pool = ctx.enter_context(tc.tile_pool(name="attn", bufs=4))
            sm_pool = ctx.enter_context(tc.tile_pool(name="small", bufs=8))
            out_pool = ctx.enter_context(tc.tile_pool(name="outt", bufs=2))

            # ---------------- helpers ----------------
            def gather_batch(dst, src_sb, src_sw):
                """Head gather dst[64par+d, hp, j, c, tl] =
                src[64(c%2)+d, c//2, 128hl + 8j + tl], reading the
                half-swapped copy when par != c%2. All on DVE (4x mode)."""
                for hl in range(HPC):
                    par, hp = hl % 2, hl // 2
                    po = 64 * par
                    for c0 in range(2):
                        srct = src_sb if par == c0 else src_sw
                        inv = srct[
                            po : po + 64, :, 128 * hl : 128 * (hl + 1)
                        ].rearrange("d a (j w) -> d j a w", w=8)
                        outv = dst[po : po + 64, hp].rearrange(
                            "d j (cc c2) w -> d j cc c2 w", c2=2
                        )[:, :, :, c0, :]
                        nc.vector.tensor_copy(outv, inv)

            def emit_swap(xt_sw, xt_out):
                nc.sync.dma_start(out=xt_sw[0:64], in_=xt_out[64:128])
                nc.sync.dma_start(out=xt_sw[64:128], in_=xt_out[0:64])

            def proj_blk(x_in, wd, xt_out, dblk, xdt, copy_eng):
                """Project one 128-col W block into xt_out[:, dblk, :]."""
                wcol = w_pool.tile([128, 8, 128], xdt, tag="wcol")
                nc.sync.dma_start(
                    out=wcol,
                    in_=wd[:, 128 * dblk : 128 * (dblk + 1)].rearrange(
                        "(a p) d -> p a d", p=128
                    ),
                )
                ps = psA.tile([128, ROWS], F32, tag="psA")
                for mt in range(8):
                    nc.tensor.matmul(
                        ps,
                        lhsT=wcol[:, mt, :],
                        rhs=x_in[:, mt, :],
                        start=(mt == 0),
                        stop=(mt == 7),
                    )
                if copy_eng is nc.scalar:
                    copy_eng.copy(xt_out[:, dblk, :], ps)
                else:
                    copy_eng.tensor_copy(xt_out[:, dblk, :], ps)

            mask_rr = [0]

            def st_pair(qc, hp, kt):
                """S^T matmuls for both heads of pair hp (concurrent row
                groups) + one fused 2-bank exp into PT[qc%2]; mask on the
                diagonal tiles (alternating DVE / GpSimd)."""
                qoff = max(0, 128 * kt - 512 * qc)
                pt = PT[hp]
                pss = []
                for half in range(2):
                    ho = 64 * half
                    ps = psA.tile([128, ROWS], F32, tag="psA")
                    nc.tensor.matmul(
                        ps[:, qoff:ROWS],
                        lhsT=KhT[ho : ho + 64, hp, kt, :, :],
                        rhs=QhT[
                            ho : ho + 64, hp,
                            4 * qc + qoff // 128 : 4 * (qc + 1), :, :,
                        ],
                        start=True,
                        stop=True,
                    )
                    pss.append(ps)
                for half in range(2):
                    nc.scalar.activation(
                        pt[:, half, kt, qoff:ROWS],
                        pss[half][:, qoff:ROWS],
                        EXP_FN,
                    )
                if kt >= 4 * qc:  # diagonal tile
                    for half in range(2):
                        eng = nc.vector
                        mask_rr[0] += 1
                        eng.tensor_mul(
                            pt[:, half, kt, qoff : qoff + 128],
                            pt[:, half, kt, qoff : qoff + 128],
                            mask_t,
                        )

            pv_rr = [0]

            def pv_step(qc, hp, s):
                """P@V chains for both heads of the pair at q-subtile s,
                normalization, and transpose into attT2."""
                nkt = 4 * qc + s + 1
                pt = PT[hp]
                accs = []
                for half in range(2):
                    hl = 2 * hp + half
                    acc = psPV_pool.tile(
                        [128, HD + 1], F32, tag="psPV", name="acc"
                    )
                    for kt in range(nkt):
                        nc.tensor.matmul(
                            acc,
                            lhsT=pt[:, half, kt, 128 * s : 128 * (s + 1)],
                            rhs=V_nat[:, hl, kt, :],
                            start=(kt == 0),
                            stop=(kt == nkt - 1),
                        )
                    accs.append(acc)
                recip = sm_pool.tile([128, 2], F32, tag="recip")
                nc.vector.reciprocal(recip[:, 0:1], accs[0][:, HD : HD + 1])
                nc.vector.reciprocal(recip[:, 1:2], accs[1][:, HD : HD + 1])
                attn2 = an_pool.tile([128, 128], BF16, tag="attn2")
                for half in range(2):
                    nc.vector.tensor_scalar_mul(
                        attn2[:, 64 * half : 64 * (half + 1)],
                        accs[half][:, 0:HD],
                        recip[:, half : half + 1],
                    )
                ps_t = psT.tile([128, 128], BF16, tag="psT")
                nc.tensor.transpose(ps_t, attn2, ident)
                nc.vector.tensor_copy(attT2[:, hp, 4 * qc + s, :], ps_t)

            def outproj(qt):
                """Output projection for one 128-row q tile; bf16 partial."""
                ot = out_pool.tile([128, 2, ROWS], BF16, tag="out_t")
                for ec in range(2):
                    po = psA.tile([128, ROWS], F32, tag="psA")
                    for pair in range(2):
                        nc.tensor.matmul(
                            po,
                            lhsT=attT2[:, pair, qt, :],
                            rhs=Wp_sb[:, pair, 512 * ec : 512 * (ec + 1)],
                            start=(pair == 0),
                            stop=(pair == 1),
                        )
                    nc.vector.tensor_copy(ot[:, ec, :], po)
                # un-permute rows: partition w=8c+tl -> row 16*tl+c.
                dst = out_d[128 * qt : 128 * (qt + 1), :].rearrange(
                    "(tl c) e -> c tl e", tl=8
                )
                nc.sync.dma_start(out=dst, in_=ot)

            def v_transpose_group(hl_lo, jg):
                """Transpose V_pre -> V_nat for heads (hl_lo, hl_lo+1) and
                j in (2jg, 2jg+1): 4 row-group-paired PE transposes."""
                for sidx in range(4):
                    j = 2 * jg + sidx // 2
                    hl = hl_lo + sidx % 2
                    hp, ho = hl // 2, (hl % 2) * 64
                    ps_v = psT.tile([128, HD], BF16, tag="psT")
                    nc.tensor.transpose(
                        ps_v,
                        V_pre[ho : ho + 64, hp, j, :, :],
                        ident[ho : ho + 64, ho : ho + 64],
                    )
                    nc.vector.tensor_copy(V_nat[:, hl, j, 0:HD], ps_v)

            # ---------------- emission ----------------
            # K projection (scalar copies), swap, gather
            x_k = xin_pool.tile([128, 8, ROWS], F16, tag="x_in")
            nc.sync.dma_start(
                out=x_k, in_=kT_d.rearrange("(a p) t -> p a t", p=128)
            )
            for dblk in range(8):
                proj_blk(x_k, Wk_d, KT_sb, dblk, F16, nc.scalar)
            emit_swap(KT_sw, KT_sb)
            gather_batch(KhT, KT_sb, KT_sw)

            # Q projection (scalar copies), swap, gather
            x_q = xin_pool.tile([128, 8, ROWS], F16, tag="x_in")
            nc.sync.dma_start(
                out=x_q, in_=qT_d.rearrange("(a p) t -> p a t", p=128)
            )
            for dblk in range(8):
                proj_blk(x_q, Wq_d, QT_sb, dblk, F16, nc.scalar)
            emit_swap(QT_sw, QT_sb)
            gather_batch(QhT, QT_sb, QT_sw)

            # V projection (vector copies) interleaved with S^T(qc=3,hp=0)
            x_v = xin_pool.tile([128, 8, ROWS], BF16, tag="x_in")
            nc.sync.dma_start(
                out=x_v, in_=vT_d.rearrange("(a p) t -> p a t", p=128)
            )
            for dblk in range(4):
                proj_blk(x_v, Wv_d, VT_sb, dblk, BF16, nc.vector)
            for kt in range(6):
                st_pair(3, 0, kt)
            for dblk in range(4, 6):
                proj_blk(x_v, Wv_d, VT_sb, dblk, BF16, nc.vector)
            for kt in range(6, 12):
                st_pair(3, 0, kt)
            for dblk in range(6, 8):
                proj_blk(x_v, Wv_d, VT_sb, dblk, BF16, nc.vector)
            for kt in range(12, 16):
                st_pair(3, 0, kt)
            emit_swap(VT_sw, VT_sb)
            gather_batch(V_pre, VT_sb, VT_sw)

            for kt in range(16):
                st_pair(3, 1, kt)

            # V transposes into V_nat (PE, row-group paired)
            for hl_lo in (0, 2):
                for jg in range(8):
                    v_transpose_group(hl_lo, jg)

            if trunc == "proj":
                nc.compile()
                return nc

            # Attention pipeline, qc descending. PT is double-buffered by
            # head pair: S^T(qc-1, hp) re-fills PT[hp] right after
            # PV(qc, hp) drains it, so exp of the next chunk overlaps the
            # current chunk's PV/out-proj work on the PE.
            for s in range(4):
                pv_step(3, 0, s)
            if trunc == "pv1":
                nc.compile()
                return nc
            for kt in range(12):
                st_pair(2, 0, kt)
            for s in range(4):
                pv_step(3, 1, s)
            for kt in range(12):
                st_pair(2, 1, kt)
            for qt in range(12, 16):
                outproj(qt)

            for s in range(4):
                pv_step(2, 0, s)
            for kt in range(8):
                st_pair(1, 0, kt)
            for s in range(4):
                pv_step(2, 1, s)
            for kt in range(8):
                st_pair(1, 1, kt)
            for qt in range(8, 12):
                outproj(qt)

            for s in range(4):
                pv_step(1, 0, s)
            for kt in range(4):
                st_pair(0, 0, kt)
            for s in range(4):
                pv_step(1, 1, s)
            for kt in range(4):
                st_pair(0, 1, kt)
            for qt in range(4, 8):
                outproj(qt)

            for s in range(4):
                pv_step(0, 0, s)
            for s in range(4):
                pv_step(0, 1, s)
            for qt in range(4):
                outproj(qt)

            if debug_dump:
                nc.sync.dma_start(out=dbg["dKT"], in_=KT_sb)
                nc.sync.dma_start(out=dbg["dQhT"], in_=QhT)
                nc.sync.dma_start(out=dbg["dKhT"], in_=KhT)
                nc.sync.dma_start(out=dbg["dVnat"], in_=V_nat)
                nc.sync.dma_start(out=dbg["dPT0"], in_=PT[0])
                nc.sync.dma_start(out=dbg["dPT1"], in_=PT[1])
                nc.sync.dma_start(out=dbg["dattT2"], in_=attT2)

    nc.compile()
    return nc


def get_program(debug_dump=False, trunc=None):
    global _PROGRAM
    if _PROGRAM is None:
        _PROGRAM = _build_program(debug_dump, trunc)
    return _PROGRAM


def make_in_maps(q, k, v, Wq, Wk, Wv, Wp):
    mask = _perm_mask_np()
    Wq_b = np.asarray(Wq, np.float32).astype(F16_NP)
    Wk_b = np.asarray(Wk, np.float32).astype(F16_NP)
    Wv_b = np.asarray(Wv, np.float32).astype(BF16_NP)
    Wp_f = np.asarray(Wp, np.float32)
    in_maps = []
    for core in range(N_CORES):
        b, g = divmod(core, 4)
        rows = slice(ROWS * g, ROWS * (g + 1))
        in_maps.append(
            {
                "qT": np.ascontiguousarray(
                    np.asarray(q[b], np.float32)[rows].T
                ).astype(F16_NP),
                "kT": np.ascontiguousarray(
                    np.asarray(k[b], np.float32)[rows].T
                ).astype(F16_NP),
                "vT": np.ascontiguousarray(
                    np.asarray(v[b], np.float32)[rows].T
                ).astype(BF16_NP),
                "Wq": Wq_b,
                "Wk": Wk_b,
                "Wv": Wv_b,
                "Wp": np.ascontiguousarray(
                    Wp_f[HPC * HD * g : HPC * HD * (g + 1)]
                ).astype(BF16_NP),
                "mask": mask,
            }
        )
    return in_maps


def kernel(q, k, v, Wq, Wk, Wv, Wp, _trace=False, _trace_kwargs=None):
    nc = get_program()
    in_maps = make_in_maps(q, k, v, Wq, Wk, Wv, Wp)
    res = run_bass_kernel_spmd(
        nc,
        in_maps,
        core_ids=list(range(N_CORES)),
        trace=_trace,
        **(_trace_kwargs or {}),
    )
    outs = [
        np.asarray(res.results[c]["out"], np.float32) for c in range(N_CORES)
    ]
    full = np.stack(
        [
            outs[0] + outs[1] + outs[2] + outs[3],
            outs[4] + outs[5] + outs[6] + outs[7],
        ]
    ).astype(np.float32)
    if _trace:
        kernel._last_result = res
    return full


# revision 34
# speedup vs baseline: 1.1393x; 1.0218x over previous
"""MultiHeadAttention (B=2, S=2048, D=1024, 16 heads, causal, torch-.view head
split) on 8 TRN2 NeuronCores.

Sharding: core c handles batch b = c//4 and heads [4g, 4g+4) with g = c%4
(head h only touches token rows [128h, 128(h+1)) of its batch, so each core
needs just 512 rows of q/k/v). Wp is row-sharded by head; each core returns a
partial (2048, 1024) output (bf16) and the host sums the 4 partials per batch
in f32.

Layout notes:
- Head h's (2048, 64) matrices come from the (128 tokens x 1024 cols) block
  via s = 16*t + c, d = col%64, c = col//64. On-chip we keep head-space
  sequence order PERMUTED within each 128-tile: w = 8*c + t_lo (t = 8*j+t_lo),
  which makes all gather copies contiguous while preserving the causal
  block structure. The final output DMA un-permutes.
- Scheduling: attention q-chunks run in DESCENDING order so the scalar
  engine's exp work (the secondary bottleneck) starts as early as possible,
  overlapped with the V projection which is interleaved into the PE stream.
- S^T matmuls for the two heads of a pair use disjoint partition halves and
  execute concurrently on the PE (row_grp h0/h64); their exps are fused into
  one 2-bank-wide activation.
"""

import numpy as np
import ml_dtypes
from contextlib import ExitStack

import concourse.bass as bass
import concourse.tile as tile
from concourse import bacc, mybir
from concourse.bass_utils import run_bass_kernel_spmd
from concourse.masks import make_identity

F32 = mybir.dt.float32
F16 = mybir.dt.float16
F16_NP = np.float16
BF16 = mybir.dt.bfloat16
BF16_NP = ml_dtypes.bfloat16

B, S, D, NH, HD = 2, 2048, 1024, 16, 64
HPC = 4          # heads per core
ROWS = 512       # token rows per core
N_CORES = 8
EXP_FN = mybir.ActivationFunctionType.Exp


def _perm_mask_np():
    """(128,128) bf16 mask in permuted within-tile coords: mask[wk, wq] = 1
    iff s(wq) >= s(wk), with s(w) = 16*(w%8) + w//8."""
    w = np.arange(128)
    s = 16 * (w % 8) + w // 8
    m = (s[None, :] >= s[:, None]).astype(np.float32)
    return m.astype(BF16_NP)


_PROGRAM = None


def _build_program(debug_dump=False, trunc=None):
    nc = bacc.Bacc("TRN2", target_bir_lowering=False, debug=False)

    qT_d = nc.dram_tensor("qT", [D, ROWS], F16, kind="ExternalInput").ap()
    kT_d = nc.dram_tensor("kT", [D, ROWS], F16, kind="ExternalInput").ap()
    vT_d = nc.dram_tensor("vT", [D, ROWS], BF16, kind="ExternalInput").ap()
    Wq_d = nc.dram_tensor("Wq", [D, D], F16, kind="ExternalInput").ap()
    Wk_d = nc.dram_tensor("Wk", [D, D], F16, kind="ExternalInput").ap()
    Wv_d = nc.dram_tensor("Wv", [D, D], BF16, kind="ExternalInput").ap()
    Wp_d = nc.dram_tensor("Wp", [HPC * HD, D], BF16, kind="ExternalInput").ap()
    mask_d = nc.dram_tensor("mask", [128, 128], BF16, kind="ExternalInput").ap()
    out_d = nc.dram_tensor("out", [S, D], BF16, kind="ExternalOutput").ap()
    dbg = {}
    if debug_dump:
        for nm, shape, dt in [
            ("dKT", [128, 8, ROWS], F16),
            ("dQhT", [128, 2, 16, 16, 8], F16),
            ("dKhT", [128, 2, 16, 16, 8], F16),
            ("dVnat", [128, HPC, 16, HD + 1], BF16),
            ("dPT0", [128, 2, 16, ROWS], BF16),
            ("dPT1", [128, 2, 16, ROWS], BF16),
            ("dattT2", [128, 2, 16, 128], BF16),
        ]:
            dbg[nm] = nc.dram_tensor(nm, shape, dt, kind="ExternalOutput").ap()

    with tile.TileContext(nc) as tc:
        with ExitStack() as ctx:
            # ---------------- persistent SBUF ----------------
            pers = ctx.enter_context(tc.tile_pool(name="pers", bufs=1))
            # projected X^T, block layout: [p, dblk, t] = X^T[128*dblk+p, t]
            QT_sb = pers.tile([128, 8, ROWS], F16, tag="QT_sb")
            KT_sb = pers.tile([128, 8, ROWS], F16, tag="KT_sb")
            VT_sb = pers.tile([128, 8, ROWS], BF16, tag="VT_sb")
            # partition-half-swapped copies
            QT_sw = pers.tile([128, 8, ROWS], F16, tag="QT_sw")
            KT_sw = pers.tile([128, 8, ROWS], F16, tag="KT_sw")
            VT_sw = pers.tile([128, 8, ROWS], BF16, tag="VT_sw")
            # head-gathered, pair-packed: [64*(h%2)+d, h//2, j, c, t_lo]
            QhT = pers.tile([128, 2, 16, 16, 8], F16, tag="QhT")
            KhT = pers.tile([128, 2, 16, 16, 8], F16, tag="KhT")
            V_pre = pers.tile([128, 2, 16, 16, 8], BF16, tag="V_pre")
            # V natural per head + ones column: [w, hl, j, 0:65]
            V_nat = pers.tile([128, HPC, 16, HD + 1], BF16, tag="V_nat")
            # exp'd S^T, double-buffered by qc parity: [kpos, half, kt, q]
            PT = [
                pers.tile(
                    [128, 2, 16, ROWS], BF16, tag=f"PT{i}", name=f"PT{i}"
                )
                for i in range(2)
            ]
            # att^T pair-packed for Wp: [64*(h%2)+d, h//2, qt, wq]
            attT2 = pers.tile([128, 2, 16, 128], BF16, tag="attT2")
            Wp_sb = pers.tile([128, 2, D], BF16, tag="Wp_sb")
            mask_t = pers.tile([128, 128], BF16, tag="mask_t")
            ident = pers.tile([128, 128], BF16, tag="ident")
            make_identity(nc, ident)

            # Non-weight DMAs go through the Pool engine's DGE so the SP
            # hardware-queue rotation carries only the 24 wcol DMAs: with
            # wcol bufs=8 each slot-reuse DMA lands on the same queue as its
            # predecessor, keeping sync-wait counts within the HW limit.
            nc.gpsimd.dma_start(out=mask_t, in_=mask_d)
            nc.gpsimd.dma_start(
                out=Wp_sb, in_=Wp_d.rearrange("(a p) e -> p a e", p=128)
            )
            nc.gpsimd.memset(V_nat[:, :, :, HD : HD + 1], 1.0)

            # ---------------- PSUM pools ----------------
            # psA: [128,512] f32 single-bank tiles: projections, S^T halves,
            # out-proj. psPV: one bank, 4 rotating [128,65] accumulators.
            # psT: one bank, transposes (V groups + att^T).
            psA = ctx.enter_context(tc.tile_pool(name="psA", bufs=5, space="PSUM"))
            psPV_pool = ctx.enter_context(
                tc.tile_pool(name="psPV", bufs=2, space="PSUM")
            )
            psT = ctx.enter_context(tc.tile_pool(name="psT", bufs=1, space="PSUM"))

            xin_pool = ctx.enter_context(tc.tile_pool(name="xin", bufs=2))
            w_pool = ctx.enter_context(tc.tile_pool(name="wcol", bufs=8))
            an_pool = ctx.enter_context(tc.tile_pool(name="attn", bufs=4))
            sm_pool = ctx.enter_context(tc.tile_pool(name="small", bufs=8))
            out_pool = ctx.enter_context(tc.tile_pool(name="outt", bufs=2))

            # ---------------- helpers ----------------
            def gather_batch(dst, src_sb, src_sw):
                """Head gather dst[64par+d, hp, j, c, tl] =
                src[64(c%2)+d, c//2, 128hl + 8j + tl], reading the
                half-swapped copy when par != c%2. All on DVE (4x mode)."""
                for hl in range(HPC):
                    par, hp = hl % 2, hl // 2
                    po = 64 * par
                    for c0 in range(2):
                        srct = src_sb if par == c0 else src_sw
                        inv = srct[
                            po : po + 64, :, 128 * hl : 128 * (hl + 1)
                        ].rearrange("d a (j w) -> d j a w", w=8)
                        outv = dst[po : po + 64, hp].rearrange(
                            "d j (cc c2) w -> d j cc c2 w", c2=2
                        )[:, :, :, c0, :]
                        nc.vector.tensor_copy(outv, inv)

            def emit_swap(xt_sw, xt_out, dblk=None):
                if dblk is None:
                    a, b = 0, 8
                else:
                    a, b = dblk, dblk + 1
                nc.gpsimd.dma_start(
                    out=xt_sw[0:64, a:b], in_=xt_out[64:128, a:b]
                )
                nc.gpsimd.dma_start(
                    out=xt_sw[64:128, a:b], in_=xt_out[0:64, a:b]
                )

            def proj_blk(x_in, wd, xt_out, xt_sw, dblk, xdt, copy_eng):
                """Project one 128-col W block into xt_out[:, dblk, :]."""
                wcol = w_pool.tile([128, 8, 128], xdt, tag="wcol")
                nc.sync.dma_start(
                    out=wcol,
                    in_=wd[:, 128 * dblk : 128 * (dblk + 1)].rearrange(
                        "(a p) d -> p a d", p=128
                    ),
                )
                ps = psA.tile([128, ROWS], F32, tag="psA")
                for mt in range(8):
                    nc.tensor.matmul(
                        ps,
                        lhsT=wcol[:, mt, :],
                        rhs=x_in[:, mt, :],
                        start=(mt == 0),
                        stop=(mt == 7),
                    )
                if copy_eng is nc.scalar:
                    copy_eng.copy(xt_out[:, dblk, :], ps)
                else:
                    copy_eng.tensor_copy(xt_out[:, dblk, :], ps)
                emit_swap(xt_sw, xt_out, dblk)

            mask_rr = [0]

            def st_pair(qc, hp, kt):
                """S^T matmuls for both heads of pair hp (concurrent row
                groups) + one fused 2-bank exp into PT[qc%2]; mask on the
                diagonal tiles (alternating DVE / GpSimd)."""
                qoff = max(0, 128 * kt - 512 * qc)
                pt = PT[hp]
                pss = []
                for half in range(2):
                    ho = 64 * half
                    ps = psA.tile([128, ROWS], F32, tag="psA")
                    nc.tensor.matmul(
                        ps[:, qoff:ROWS],
                        lhsT=KhT[ho : ho + 64, hp, kt, :, :],
                        rhs=QhT[
                            ho : ho + 64, hp,
                            4 * qc + qoff // 128 : 4 * (qc + 1), :, :,
                        ],
                        start=True,
                        stop=True,
                    )
                    pss.append(ps)
                for half in range(2):
                    nc.scalar.activation(
                        pt[:, half, kt, qoff:ROWS],
                        pss[half][:, qoff:ROWS],
                        EXP_FN,
                    )
                if kt >= 4 * qc:  # diagonal tile
                    for half in range(2):
                        eng = nc.vector if mask_rr[0] % 2 == 0 else nc.gpsimd
                        mask_rr[0] += 1
                        eng.tensor_mul(
                            pt[:, half, kt, qoff : qoff + 128],
                            pt[:, half, kt, qoff : qoff + 128],
                            mask_t,
                        )

            pv_rr = [0]

            def pv_step(qc, hp, s):
                """P@V chains for both heads of the pair at q-subtile s,
                normalization, and transpose into attT2."""
                nkt = 4 * qc + s + 1
                pt = PT[hp]
                accs = []
                for half in range(2):
                    hl = 2 * hp + half
                    acc = psPV_pool.tile(
                        [128, HD + 1], F32, tag="psPV", name="acc"
                    )
                    for kt in range(nkt):
                        nc.tensor.matmul(
                            acc,
                            lhsT=pt[:, half, kt, 128 * s : 128 * (s + 1)],
                            rhs=V_nat[:, hl, kt, :],
                            start=(kt == 0),
                            stop=(kt == nkt - 1),
                        )
                    accs.append(acc)
                recip = sm_pool.tile([128, 2], F32, tag="recip")
                nc.vector.reciprocal(recip[:, 0:1], accs[0][:, HD : HD + 1])
                nc.vector.reciprocal(recip[:, 1:2], accs[1][:, HD : HD + 1])
                attn2 = an_pool.tile([128, 128], BF16, tag="attn2")
                for half in range(2):
                    nc.vector.tensor_scalar_mul(
                        attn2[:, 64 * half : 64 * (half + 1)],
                        accs[half][:, 0:HD],
                        recip[:, half : half + 1],
                    )
                return attn2

            def pv_finish(qc, hp, s, attn2):
                ps_t = psT.tile([128, 128], BF16, tag="psT")
                nc.tensor.transpose(ps_t, attn2, ident)
                nc.vector.tensor_copy(attT2[:, hp, 4 * qc + s, :], ps_t)

            def outproj(qt):
                """Output projection for one 128-row q tile; bf16 partial."""
                ot = out_pool.tile([128, 2, ROWS], BF16, tag="out_t")
                for ec in range(2):
                    po = psA.tile([128, ROWS], F32, tag="psA")
                    for pair in range(2):
                        nc.tensor.matmul(
                            po,
                            lhsT=attT2[:, pair, qt, :],
                            rhs=Wp_sb[:, pair, 512 * ec : 512 * (ec + 1)],
                            start=(pair == 0),
                            stop=(pair == 1),
                        )
                    nc.vector.tensor_copy(ot[:, ec, :], po)
                # un-permute rows: partition w=8c+tl -> row 16*tl+c.
                dst = out_d[128 * qt : 128 * (qt + 1), :].rearrange(
                    "(tl c) e -> c tl e", tl=8
                )
                nc.sync.dma_start(out=dst, in_=ot)

            def v_transpose_group(hl_lo, jg):
                """Transpose V_pre -> V_nat for heads (hl_lo, hl_lo+1) and
                j in (2jg, 2jg+1): 4 row-group-paired PE transposes."""
                for sidx in range(4):
                    j = 2 * jg + sidx // 2
                    hl = hl_lo + sidx % 2
                    hp, ho = hl // 2, (hl % 2) * 64
                    ps_v = psT.tile([128, HD], BF16, tag="psT")
                    nc.tensor.transpose(
                        ps_v,
                        V_pre[ho : ho + 64, hp, j, :, :],
                        ident[ho : ho + 64, ho : ho + 64],
                    )
                    nc.vector.tensor_copy(V_nat[:, hl, j, 0:HD], ps_v)

            # ---------------- emission ----------------
            # K projection (scalar copies), swap, gather
            x_k = xin_pool.tile([128, 8, ROWS], F16, tag="x_in")
            nc.gpsimd.dma_start(
                out=x_k, in_=kT_d.rearrange("(a p) t -> p a t", p=128)
            )
            for dblk in range(8):
                proj_blk(x_k, Wk_d, KT_sb, KT_sw, dblk, F16, nc.scalar)
            gather_batch(KhT, KT_sb, KT_sw)

            # Q projection (scalar copies), swap, gather
            x_q = xin_pool.tile([128, 8, ROWS], F16, tag="x_in")
            nc.gpsimd.dma_start(
                out=x_q, in_=qT_d.rearrange("(a p) t -> p a t", p=128)
            )
            for dblk in range(8):
                proj_blk(x_q, Wq_d, QT_sb, QT_sw, dblk, F16, nc.scalar)
            gather_batch(QhT, QT_sb, QT_sw)

            # V projection (vector copies) interleaved with S^T(qc=3,hp=0)
            x_v = xin_pool.tile([128, 8, ROWS], BF16, tag="x_in")
            nc.gpsimd.dma_start(
                out=x_v, in_=vT_d.rearrange("(a p) t -> p a t", p=128)
            )
            for dblk in range(4):
                proj_blk(x_v, Wv_d, VT_sb, VT_sw, dblk, BF16, nc.vector)
            for kt in range(6):
                st_pair(3, 0, kt)
            for dblk in range(4, 6):
                proj_blk(x_v, Wv_d, VT_sb, VT_sw, dblk, BF16, nc.vector)
            for kt in range(6, 12):
                st_pair(3, 0, kt)
            for dblk in range(6, 8):
                proj_blk(x_v, Wv_d, VT_sb, VT_sw, dblk, BF16, nc.vector)
            for kt in range(12, 16):
                st_pair(3, 0, kt)
            gather_batch(V_pre, VT_sb, VT_sw)

            for kt in range(16):
                st_pair(3, 1, kt)

            # V transposes into V_nat (PE, row-group paired)
            for hl_lo in (0, 2):
                for jg in range(8):
                    v_transpose_group(hl_lo, jg)

            if trunc == "proj":
                nc.compile()
                return nc

            # Attention pipeline, qc descending. PT is double-buffered by
            # head pair: S^T(qc-1, hp) re-fills PT[hp] right after
            # PV(qc, hp) drains it, so exp of the next chunk overlaps the
            # current chunk's PV/out-proj work on the PE.
            def pv_phase(qc, hp, st_qc=None, st_n=0):
                # all chains first (they read PT[hp]); only then the next
                # chunk's S^T pairs (whose exp overwrites PT[hp]); the
                # transposes last so the PE never waits on the vector norm
                # chain of the most recent subtile.
                a2 = [pv_step(qc, hp, s) for s in range(4)]
                if st_qc is not None:
                    for kt in range(st_n):
                        st_pair(st_qc, hp, kt)
                for s in range(4):
                    pv_finish(qc, hp, s, a2[s])

            pv_phase(3, 0, 2, 12)
            pv_phase(3, 1, 2, 12)
            for qt in range(12, 16):
                outproj(qt)
            pv_phase(2, 0, 1, 8)
            pv_phase(2, 1, 1, 8)
            for qt in range(8, 12):
                outproj(qt)
            pv_phase(1, 0, 0, 4)
            pv_phase(1, 1, 0, 4)
            for qt in range(4, 8):
                outproj(qt)
            pv_phase(0, 0)
            pv_phase(0, 1)
            for qt in range(4):
                outproj(qt)

            if debug_dump:
                nc.sync.dma_start(out=dbg["dKT"], in_=KT_sb)
                nc.sync.dma_start(out=dbg["dQhT"], in_=QhT)
                nc.sync.dma_start(out=dbg["dKhT"], in_=KhT)
                nc.sync.dma_start(out=dbg["dVnat"], in_=V_nat)
                nc.sync.dma_start(out=dbg["dPT0"], in_=PT[0])
                nc.sync.dma_start(out=dbg["dPT1"], in_=PT[1])
                nc.sync.dma_start(out=dbg["dattT2"], in_=attT2)

    nc.compile()
    return nc


def get_program(debug_dump=False, trunc=None):
    global _PROGRAM
    if _PROGRAM is None:
        _PROGRAM = _build_program(debug_dump, trunc)
    return _PROGRAM


def make_in_maps(q, k, v, Wq, Wk, Wv, Wp):
    mask = _perm_mask_np()
    Wq_b = np.asarray(Wq, np.float32).astype(F16_NP)
    Wk_b = np.asarray(Wk, np.float32).astype(F16_NP)
    Wv_b = np.asarray(Wv, np.float32).astype(BF16_NP)
    Wp_f = np.asarray(Wp, np.float32)
    in_maps = []
    for core in range(N_CORES):
        b, g = divmod(core, 4)
        rows = slice(ROWS * g, ROWS * (g + 1))
        in_maps.append(
            {
                "qT": np.ascontiguousarray(
                    np.asarray(q[b], np.float32)[rows].T
                ).astype(F16_NP),
                "kT": np.ascontiguousarray(
                    np.asarray(k[b], np.float32)[rows].T
                ).astype(F16_NP),
                "vT": np.ascontiguousarray(
                    np.asarray(v[b], np.float32)[rows].T
                ).astype(BF16_NP),
                "Wq": Wq_b,
                "Wk": Wk_b,
                "Wv": Wv_b,
                "Wp": np.ascontiguousarray(
                    Wp_f[HPC * HD * g : HPC * HD * (g + 1)]
                ).astype(BF16_NP),
                "mask": mask,
            }
        )
    return in_maps


def kernel(q, k, v, Wq, Wk, Wv, Wp, _trace=False, _trace_kwargs=None):
    nc = get_program()
    in_maps = make_in_maps(q, k, v, Wq, Wk, Wv, Wp)
    res = run_bass_kernel_spmd(
        nc,
        in_maps,
        core_ids=list(range(N_CORES)),
        trace=_trace,
        **(_trace_kwargs or {}),
    )
    outs = [
        np.asarray(res.results[c]["out"], np.float32) for c in range(N_CORES)
    ]
    full = np.stack(
        [
            outs[0] + outs[1] + outs[2] + outs[3],
            outs[4] + outs[5] + outs[6] + outs[7],
        ]
    ).astype(np.float32)
    if _trace:
        kernel._last_result = res
    return full
